# revision 18
# baseline (speedup 1.0000x reference)
"""GAT multi-head attention layer (nn_GATMutiHeadAttLayer) on 8 Trainium2 cores.

Head-sharded: core h computes head h entirely (no collectives).

Math (per head):
  h = X @ W                       [S, FOUT]
  f1 = h @ a1, f2 = h @ a2        [S]
  e[i,j] = lrelu(f1[i] + f2[j], 0.2), masked by adj[i,j]
  attn = softmax(e, axis=i)  (denominator s[j] = sum_i)
  out = attn @ h, concat heads, ELU.

Device formulation (transposed, j on partitions; fp16 tiles).  All per-band
elementwise ops are chosen for the DVE's fast uop modes (TS = 2 results/cyc
with per-partition scalar pair; TT = 2 results/cyc; STT/TTR are 1x and
avoided):
  exp(lrelu(z)) = max(exp(z), exp(0.2 z)),  z = f1[i] + f2[j]
  u'[j,i] = max(R1b[i] * eu[j], ev[j])          (TS: mult, max)
     R1b = exp(0.8 f1) bcast, eu = exp(f2), ev = exp(0.2 f2)
  q[j,i]  = u' * E1sb[i]                        (TT: mult)
     E1sb = exp(0.2 f1) bcast;  q = exp(lrelu(z)) unmasked, >= 0
  pb = q + mask'  (mask' in {0, -BIG} fp16, DMA'd)   (TT: add)
  p = relu(pb), s[j] = sum_i p   (one ACT pass: zeroes masked entries AND
                                  row-sums via the fused accumulator)
  hp = h[band] * (1/s)  (DVE reciprocal + ACT copy-with-scale; deferred one
                         band so the DVE FIFO never stalls on the ACT pass)
  out^T[o,i] = sum_j hp[j,o] * p[j,i]   (PE, PSUM-accumulated over bands)
  final: ELU(out^T) -> DRAM; host transposes/concats heads.

Preamble (PE fp16):
  wa = W @ [a1|a2] via W^T; zb[p,i] = f1[i] directly via a stride-0
  broadcast stationary (wa1 replicated); R1b/E1sb = Exp(0.8/0.2 * zb)
  straight out of PSUM.  [h_band | f2_band] = xt_band^T @ [W | wa2]
  (one N=65 matmul per band);  eu = exp(f2), ev = exp(0.2 f2).

Host prep: X^T, W, W^T, [a1|a2] cast fp16; adj^T encoded {0, -60000} fp16.
All model compute (matmuls, exp, masking, softmax, ELU) runs on device.

Rejected on measurement: STT/TTR formulations (1x uop, 4.4us/band), SWDGE
accumulate-DMA masking (runtime crash), GPSIMD elementwise offload (shared
SBUF port degrades DVE fast modes 2-4x), GPSIMD normalize_recip (2us/call
on the critical chain).
"""

import contextlib
import ctypes
import os
import sys
import types
from contextlib import ExitStack

import numpy as np

import concourse.bass as bass
import concourse.tile as tile
from concourse import bacc, mybir
from concourse import bass_utils

AF = mybir.ActivationFunctionType
ALU = mybir.AluOpType
DT = mybir.dt

S = 4096
FIN = 128
FOUT = 64
H = 8
ALPHA = 0.2

LAST_RESULTS = None  # BassKernelResults of the most recent run (for test harness)

# ---------------------------------------------------------------------------
# NTFF profile hook shim: antenv.axon_hooks is absent in this container; the
# trace=True path of run_bass_kernel_spmd imports it. Recreate it via ctypes
# against libaxon_pjrt.so (same as trn_agent_boot does).
_SO_PATH = "/opt/axon/libaxon_pjrt.so"


def _make_ntff_hook():
    try:
        lib = ctypes.CDLL(_SO_PATH)
    except OSError:
        return None
    if not hasattr(lib, "axon_start_nrt_profile"):
        return None
    lib.axon_start_nrt_profile.argtypes = [ctypes.POINTER(ctypes.c_int64), ctypes.c_size_t]
    lib.axon_start_nrt_profile.restype = ctypes.c_int64
    lib.axon_stop_nrt_profile.argtypes = [ctypes.c_char_p]
    lib.axon_stop_nrt_profile.restype = ctypes.c_int64

    @contextlib.contextmanager
    def _hook(output_dir, device_ids):
        import jax

        jax.devices()
        if device_ids:
            ids = (ctypes.c_int64 * len(device_ids))(*device_ids)
            rc = lib.axon_start_nrt_profile(ids, len(device_ids))
        else:
            rc = lib.axon_start_nrt_profile(None, 0)
        if rc != 0:
            raise RuntimeError(f"axon_start_nrt_profile rc={rc}")
        try:
            yield
        finally:
            n = lib.axon_stop_nrt_profile(str(output_dir).encode())
            if n <= 0:
                print(f"ntff profile: rc={n} (no files?) dir={output_dir}", file=sys.stderr)

    return _hook


def _install_ntff_shim():
    if "antenv.axon_hooks" in sys.modules:
        return
    mod = types.ModuleType("antenv.axon_hooks")
    _hook = _make_ntff_hook()
    mod.get_axon_ntff_profile_hook = lambda: _hook
    mod.set_axon_ntff_profile_hook = lambda h: None
    sys.modules["antenv.axon_hooks"] = mod
    try:
        import antenv

        antenv.axon_hooks = mod
    except ImportError:
        pass


_install_ntff_shim()

# ---------------------------------------------------------------------------

PLAN = os.environ.get("KERNEL_PLAN", "Z")
USE_GP = os.environ.get("KERNEL_GP", "0") == "1"
# Width of the per-band mask-add slice offloaded to GPSIMD (0 = all on DVE)
GPW = int(os.environ.get("KERNEL_GPW", "0"))
# Number of B-type (ACT-heavy) bands out of nb=32.  A-band: DVE TS+TT+TT,
# ACT Relu+accum.  B-band: ACT Prelu+Exp+accum, DVE mask-TT only.
# Measured per-band costs: A = 6.29us DVE / 3.97us ACT; B = 2.68 / 7.66.
# Balance lands at ~10/32.  B bands live in [2, nb-6]: none at the start
# (ramp: prelu would queue behind the preamble exps) and none at the end
# (drain: the final A-run lets ACT catch up before the last band's chain).
NB_B = int(os.environ.get("KERNEL_NB", "10"))


def build_nc(s=S, plan=None):
    """Build + compile the per-core Bass program (same program on all cores)."""
    plan = plan or PLAN
    nb = s // 128     # number of j-bands
    nch = s // 512    # number of 512-wide i-chunks

    nc = bacc.Bacc("TRN2", target_bir_lowering=False, debug=False, enable_asserts=False)

    xt = nc.dram_tensor("xt", [FIN, s], DT.float16, kind="ExternalInput").ap()
    w = nc.dram_tensor("w", [FIN, FOUT], DT.float16, kind="ExternalInput").ap()
    wt = nc.dram_tensor("wt", [FOUT, FIN], DT.float16, kind="ExternalInput").ap()
    a12 = nc.dram_tensor("a12", [FOUT, 2], DT.float16, kind="ExternalInput").ap()
    adjt = nc.dram_tensor("adjt", [s, s], DT.float16, kind="ExternalInput").ap()
    out = nc.dram_tensor("out", [FOUT, s], DT.float16, kind="ExternalOutput").ap()

    with tile.TileContext(nc) as tc, ExitStack() as ctx:
        _body(ctx, tc, nc, xt, w, wt, a12, adjt, out, s, nb, nch, plan)

    if os.environ.get("KERNEL_LDW1", "1") == "1":
        # Mark matmuls whose stationary operand AP repeats the immediately
        # preceding matmul's as non-self-loading (PE keeps the loaded array).
        n_marked = 0
        for blk in nc.m.functions[0].blocks:
            prev_w = None
            for inst in blk.instructions:
                if type(inst).__name__ != "InstMatmult":
                    continue
                wkey = repr(inst.ins[1])
                if prev_w == wkey:
                    inst.ldweights = False
                    n_marked += 1
                prev_w = wkey
        print(f"KERNEL_LDW1: marked {n_marked} matmuls non-self-loading")

    nc.compile()
    return nc


def _body(ctx, tc, nc, xt, w, wt, a12, adjt, out, s, nb, nch, plan):
    f32, f16 = DT.float32, DT.float16

    if NB_B > 1:
        bset = {2 + round(k * (nb - 8) / (NB_B - 1)) for k in range(NB_B)}
    else:
        bset = {nb // 2} if NB_B else set()

    def is_b(b):
        """True if band b runs the ACT-heavy path."""
        return b in bset

    # ---------------- persistent intermediates (live through main loop) ----
    cpool = ctx.enter_context(tc.tile_pool(name="const", bufs=1))
    r1b_sb = cpool.tile([128, s], f16, tag="r1b")      # exp(0.8 f1[i]) bcast (A bands)
    e1sb_sb = cpool.tile([128, s], f16, tag="e1sb")    # exp(0.2 f1[i]) bcast (A bands)
    f1b_sb = cpool.tile([128, s], f16, tag="f1b")      # f1[i] bcast (B bands)
    hf_sb = cpool.tile([128, nb * (FOUT + 1)], f32, tag="hf")  # [h|f2] per band
    eu_sb = cpool.tile([128, nb], f32, tag="eu")       # exp(f2), band b in col b
    ev_sb = cpool.tile([128, nb], f32, tag="ev")       # exp(0.2 f2)

    def h_col(b):
        return hf_sb[:, b * (FOUT + 1) : b * (FOUT + 1) + FOUT]

    def f2_col(b):
        return hf_sb[:, b * (FOUT + 1) + FOUT : (b + 1) * (FOUT + 1)]

    # ---------------- preamble (scoped pools, freed before main loop) ------
    with tc.tile_pool(name="pre_sb", bufs=1) as tpool:
        # weights first: tiny transfers that gate the wa matmul
        w65_sb = tpool.tile([FIN, FOUT + 1], f16, tag="w65")
        nc.sync.dma_start(w65_sb[:, 0:FOUT], w[:])
        wt_sb = tpool.tile([FOUT, FIN], f16, tag="wt")
        nc.sync.dma_start(wt_sb[:], wt[:])
        a12_sb = tpool.tile([FOUT, 2], f16, tag="a12")
        nc.sync.dma_start(a12_sb[:], a12[:])
        xt_sb = tpool.tile([FIN, s], f16, tag="xt")
        # split across partition-groups x column-halves: a single 1MB DMA
        # moves ~128 x 8KB packets serially on ONE hw queue (~26.5GB/s,
        # ~40us latency); 8 slices land in parallel on separate queues.
        for pg in range(4):
            for ch in range(2):
                nc.sync.dma_start(
                    xt_sb[32 * pg : 32 * (pg + 1), bass.ts(ch, s // 2)],
                    xt[32 * pg : 32 * (pg + 1), bass.ts(ch, s // 2)],
                )
        wa_sb = tpool.tile([FIN, 2], f16, tag="wa")    # [wa1 | wa2]

        # wa = W @ [a1 | a2]  (contract over FOUT)
        with tc.tile_pool(name="pre_wa", bufs=1, space="PSUM") as pwa:
            wa_ps = pwa.tile([FIN, 2], f32, tag="wa")
            nc.tensor.matmul(wa_ps[:], lhsT=wt_sb[:], rhs=a12_sb[:], start=True, stop=True)
            nc.vector.tensor_copy(wa_sb[:], wa_ps[:])
            nc.vector.tensor_copy(w65_sb[:, FOUT : FOUT + 1], wa_ps[:, 1:2])

        # Row-first broadcast: one rank-1 matmul gives f1 as a [1, s] PSUM
        # row; Exp(0.8 f1)/Exp(0.2 f1)/copy produce three f16 rows (7us of
        # ACT instead of 16 chunked bcast exps), which three 1-contract
        # matmuls broadcast across all 128 partitions.
        er1_row = tpool.tile([1, s], f16, tag="er1row")
        e1s_row = tpool.tile([1, s], f16, tag="e1srow")
        f1_row = tpool.tile([1, s], f16, tag="f1row")
        ones_sb = tpool.tile([1, 128], f16, tag="ones")
        nc.vector.memset(ones_sb[:], 1.0)
        with tc.tile_pool(name="pre_fr", bufs=1, space="PSUM") as pfr:
            frow_ps = pfr.tile([1, s], f32, tag="frow")
            for k in range(s // 512):
                nc.tensor.matmul(frow_ps[:, bass.ts(k, 512)], lhsT=wa_sb[:, 0:1],
                                 rhs=xt_sb[:, bass.ts(k, 512)], start=True, stop=True)
            nc.scalar.activation(er1_row[:], frow_ps[:], AF.Exp, scale=0.8)
            nc.scalar.activation(e1s_row[:], frow_ps[:], AF.Exp, scale=0.2)
            nc.vector.tensor_copy(f1_row[:], frow_ps[:])
        with tc.tile_pool(name="pre_bc", bufs=2, space="PSUM") as pbcp:
            for row_t, dst in ((er1_row, r1b_sb), (e1s_row, e1sb_sb), (f1_row, f1b_sb)):
                for hh in range(2):
                    sl = bass.ts(hh, s // 2)
                    pbc = pbcp.tile([128, s // 2], f32, tag="bc")
                    for k in range(s // 2 // 512):
                        nc.tensor.matmul(pbc[:, bass.ts(k, 512)], lhsT=ones_sb[:],
                                         rhs=row_t[:, hh * (s // 2) + k * 512 : hh * (s // 2) + (k + 1) * 512],
                                         start=True, stop=True)
                    nc.vector.tensor_copy(dst[:, sl], pbc[:])

        # [h_band | f2_band] = xt_band^T @ [W | wa2]  (one matmul + one copy
        # per band into the combined hf tile; f2 stays at stride FOUT+1)
        with tc.tile_pool(name="pre_h", bufs=3, space="PSUM") as phf:
            for b in range(nb):
                ph = phf.tile([128, FOUT + 1], f32, tag="hf")
                nc.tensor.matmul(ph[:], lhsT=xt_sb[:, bass.ts(b, 128)], rhs=w65_sb[:], start=True, stop=True)
                nc.vector.tensor_copy(hf_sb[:, bass.ts(b, FOUT + 1)], ph[:])

        # exp of f2 cols (strided reads of the hf tile; tiny ACT ops)
        f2_strided = hf_sb[:, FOUT :: FOUT + 1]
        nc.scalar.activation(eu_sb[:], f2_strided, AF.Exp)
        nc.scalar.activation(ev_sb[:], f2_strided, AF.Exp, scale=0.2)

    # ---------------- main loop over j-bands ----------------
    upool = ctx.enter_context(tc.tile_pool(name="umax", bufs=2))
    ppool = ctx.enter_context(tc.tile_pool(name="pmat", bufs=4))
    scrpool = ctx.enter_context(tc.tile_pool(name="scr", bufs=4))
    spool = ctx.enter_context(tc.tile_pool(name="svec", bufs=12))
    hppool = ctx.enter_context(tc.tile_pool(name="hp", bufs=3))
    mpool = ctx.enter_context(tc.tile_pool(name="mask", bufs=4))
    qpool = ctx.enter_context(tc.tile_pool(name="qtmp", bufs=2))
    wpool = ctx.enter_context(tc.tile_pool(name="wlr", bufs=3))

    mainpsum = ctx.enter_context(tc.tile_pool(name="out_psum", bufs=1, space="PSUM"))
    psum_out = mainpsum.tile([FOUT, s], f32, tag="out")

    def finish_band(b, p_t, s_t, elu_emit=None):
        """Reciprocal + hp scaling + accumulating matmuls for band b.

        Deferred one iteration so the DVE FIFO's reciprocal never waits on
        the ACT passes of the same band (head-of-line stall)."""
        hp_t = hppool.tile([128, FOUT], f16, tag="hp")
        rs_t = spool.tile([128, 1], f32, tag="rs")
        nc.vector.reciprocal(rs_t[:], s_t[:])
        # hp = h * (1/s): per-partition scalar mult on DVE (cheap TS) keeps
        # the ACT queue free for the big relu/prelu/exp passes.
        nc.vector.tensor_scalar(out=hp_t[:], in0=h_col(b), scalar1=rs_t[:],
                                scalar2=None, op0=ALU.mult)
        for c in range(nch):
            nc.tensor.matmul(
                psum_out[:, bass.ts(c, 512)], lhsT=hp_t[:], rhs=p_t[:, bass.ts(c, 512)],
                start=(b == 0), stop=(b == nb - 1),
            )
            if elu_emit is not None:
                elu_emit(c)

    # w tiles (Prelu of f1+f2) for B bands, computed one band ahead
    w_tiles = {}

    def emit_prelu(b):
        w_t = wpool.tile([128, s], f16, tag="w")
        nc.scalar.activation(w_t[:], f1b_sb[:], AF.Prelu, bias=f2_col(b), alpha=0.2)
        w_tiles[b] = w_t

    # prefetch the first few band masks so band 0 never waits on DMA
    def mask_dma(b, nsplit=1):
        # nsplit>1 halves/quarters the on-queue latency (1MB = ~40us on one
        # hw queue) at a cost of ~0.6us dispatch each — only worth it for
        # the ramp-gating first masks.
        m_t = mpool.tile([128, s], f16, tag="m")
        pw = 128 // nsplit
        for pg in range(nsplit):
            nc.sync.dma_start(m_t[pw * pg : pw * (pg + 1), :],
                              adjt[b * 128 + pw * pg : b * 128 + pw * (pg + 1), :])
        return m_t

    m_tiles = {}
    for b in range(min(3, nb)):
        m_tiles[b] = mask_dma(b, nsplit=4)

    if is_b(0):
        emit_prelu(0)
    pending = None
    for b in range(nb):
        if b in m_tiles:
            m_t = m_tiles.pop(b)
        else:
            m_t = mask_dma(b)
        p_t = scrpool.tile([128, s], f16, tag="p")
        s_t = spool.tile([128, 1], f32, tag="s")

        if b + 1 < nb and is_b(b + 1):
            emit_prelu(b + 1)  # ACT fills while DVE works on band b

        if not is_b(b):
            # ---- A band (DVE-heavy): u'=max(R1b*eu,ev); q=u'*E1sb; +mask;
            #      ACT Relu zeroes masked entries and row-sums.
            u_t = upool.tile([128, s], f16, tag="u")
            nc.vector.tensor_scalar(
                out=u_t[:], in0=r1b_sb[:], scalar1=eu_sb[:, b : b + 1],
                scalar2=ev_sb[:, b : b + 1], op0=ALU.mult, op1=ALU.max,
            )
            q_t = qpool.tile([128, s], f16, tag="q")
            nc.vector.tensor_tensor(out=q_t[:], in0=u_t[:], in1=e1sb_sb[:], op=ALU.mult)
            if pending is not None:
                finish_band(*pending)
                pending = None
            pb_t = ppool.tile([128, s], f16, tag="pb")
            nc.vector.tensor_tensor(out=pb_t[:], in0=q_t[:], in1=m_t[:], op=ALU.add)
            nc.scalar.activation(p_t[:], pb_t[:], AF.Relu, accum_out=s_t[:])
        else:
            # ---- B band (ACT-heavy): pb = prelu(f1+f2) + mask; p = exp(pb)
            w_t = w_tiles.pop(b)
            pb_t = ppool.tile([128, s], f16, tag="pb")
            nc.vector.tensor_tensor(out=pb_t[:], in0=w_t[:], in1=m_t[:], op=ALU.add)
            if pending is not None:
                finish_band(*pending)
                pending = None
            nc.scalar.activation(p_t[:], pb_t[:], AF.Exp, accum_out=s_t[:])

        pending = (b, p_t, s_t)

    # ---------------- ELU + writeout, pipelined with the last band's stop-
    # matmuls: chunk c's ELU is emitted right after its final accumulation.
    fpool = ctx.enter_context(tc.tile_pool(name="fin", bufs=2))
    ew = s // nch  # 512

    def elu_emit(c):
        sl = bass.ts(c, ew)
        r_t = fpool.tile([FOUT, ew], f32, tag="relu")
        nc.scalar.activation(r_t[:], psum_out[:, sl], AF.Relu)
        mn_t = fpool.tile([FOUT, ew], f32, tag="min")
        nc.vector.tensor_scalar_min(out=mn_t[:], in0=psum_out[:, sl], scalar1=0.0)
        e_t = fpool.tile([FOUT, ew], f32, tag="exp")
        nc.scalar.activation(e_t[:], mn_t[:], AF.Exp)
        f_t = fpool.tile([FOUT, ew], f16, tag="fin")
        # f = (e - 1) + r   (f16 out halves the writeback; host upcasts)
        nc.vector.scalar_tensor_tensor(out=f_t[:], in0=e_t[:], scalar=-1.0, in1=r_t[:], op0=ALU.add, op1=ALU.add)
        nc.sync.dma_start(out[:, sl], f_t[:])

    finish_band(*pending, elu_emit=elu_emit)


_NC_CACHE = {}


def _get_nc(s=S, plan=None):
    key = (s, plan or PLAN)
    if key not in _NC_CACHE:
        _NC_CACHE[key] = build_nc(s, plan)
    return _NC_CACHE[key]


def kernel(input_seq, adj, W, a_1, a_2):
    """Full-input entry point: shards by head across 8 cores, returns [S, H*FOUT]."""
    global LAST_RESULTS
    X = np.asarray(input_seq)[0]          # [S, FIN] f32
    adjm = np.asarray(adj)[0]             # [S, S] int32
    Wn = np.asarray(W)                    # [H, FIN, FOUT]
    a1n = np.asarray(a_1)                 # [H, FOUT, 1]
    a2n = np.asarray(a_2)                 # [H, FOUT, 1]

    s = X.shape[0]
    xt = np.ascontiguousarray(X.T, dtype=np.float16)
    # mask encoded as {0, -BIG}: p = relu(p + mask') zeroes masked-out entries
    adjt = np.where(np.ascontiguousarray(adjm.T) != 0, np.float16(0.0), np.float16(-60000.0))

    nc = _get_nc(s)
    in_maps = [
        {
            "xt": xt,
            "w": np.ascontiguousarray(Wn[h], dtype=np.float16),
            "wt": np.ascontiguousarray(Wn[h].T, dtype=np.float16),
            "a12": np.ascontiguousarray(
                np.concatenate([a1n[h], a2n[h]], axis=1), dtype=np.float16
            ),
            "adjt": adjt,
        }
        for h in range(H)
    ]
    res = bass_utils.run_bass_kernel_spmd(nc, in_maps, core_ids=list(range(H)))
    LAST_RESULTS = res

    outf = np.empty((s, H * FOUT), dtype=np.float32)
    for h in range(H):
        outf[:, h * FOUT : (h + 1) * FOUT] = res.results[h]["out"].T
    return outf



# revision 19
# speedup vs baseline: 1.0525x; 1.0525x over previous
"""GAT multi-head attention layer (nn_GATMutiHeadAttLayer) on 8 Trainium2 cores.

Head-sharded: core h computes head h entirely (no collectives).

Math (per head):
  h = X @ W                       [S, FOUT]
  f1 = h @ a1, f2 = h @ a2        [S]
  e[i,j] = lrelu(f1[i] + f2[j], 0.2), masked by adj[i,j]
  attn = softmax(e, axis=i)  (denominator s[j] = sum_i)
  out = attn @ h, concat heads, ELU.

Device formulation (transposed, j on partitions; fp16 tiles).  All per-band
elementwise ops are chosen for the DVE's fast uop modes (TS = 2 results/cyc
with per-partition scalar pair; TT = 2 results/cyc; STT/TTR are 1x and
avoided):
  exp(lrelu(z)) = max(exp(z), exp(0.2 z)),  z = f1[i] + f2[j]
  u'[j,i] = max(R1b[i] * eu[j], ev[j])          (TS: mult, max)
     R1b = exp(0.8 f1) bcast, eu = exp(f2), ev = exp(0.2 f2)
  q[j,i]  = u' * E1sb[i]                        (TT: mult)
     E1sb = exp(0.2 f1) bcast;  q = exp(lrelu(z)) unmasked, >= 0
  pb = q + mask'  (mask' in {0, -BIG} fp16, DMA'd)   (TT: add)
  p = relu(pb), s[j] = sum_i p   (one ACT pass: zeroes masked entries AND
                                  row-sums via the fused accumulator)
  hp = h[band] * (1/s)  (DVE reciprocal + ACT copy-with-scale; deferred one
                         band so the DVE FIFO never stalls on the ACT pass)
  out^T[o,i] = sum_j hp[j,o] * p[j,i]   (PE, PSUM-accumulated over bands)
  final: ELU(out^T) -> DRAM; host transposes/concats heads.

Preamble (PE fp16):
  wa = W @ [a1|a2] via W^T; zb[p,i] = f1[i] directly via a stride-0
  broadcast stationary (wa1 replicated); R1b/E1sb = Exp(0.8/0.2 * zb)
  straight out of PSUM.  [h_band | f2_band] = xt_band^T @ [W | wa2]
  (one N=65 matmul per band);  eu = exp(f2), ev = exp(0.2 f2).

Host prep: X^T, W, W^T, [a1|a2] cast fp16; adj^T encoded {0, -60000} fp16.
All model compute (matmuls, exp, masking, softmax, ELU) runs on device.

Rejected on measurement: STT/TTR formulations (1x uop, 4.4us/band), SWDGE
accumulate-DMA masking (runtime crash), GPSIMD elementwise offload (shared
SBUF port degrades DVE fast modes 2-4x), GPSIMD normalize_recip (2us/call
on the critical chain).
"""

import contextlib
import ctypes
import os
import sys
import types
from contextlib import ExitStack

import numpy as np

import concourse.bass as bass
import concourse.tile as tile
from concourse import bacc, mybir
from concourse import bass_utils

AF = mybir.ActivationFunctionType
ALU = mybir.AluOpType
DT = mybir.dt

S = 4096
FIN = 128
FOUT = 64
H = 8
ALPHA = 0.2

LAST_RESULTS = None  # BassKernelResults of the most recent run (for test harness)

# ---------------------------------------------------------------------------
# NTFF profile hook shim: antenv.axon_hooks is absent in this container; the
# trace=True path of run_bass_kernel_spmd imports it. Recreate it via ctypes
# against libaxon_pjrt.so (same as trn_agent_boot does).
_SO_PATH = "/opt/axon/libaxon_pjrt.so"


def _make_ntff_hook():
    try:
        lib = ctypes.CDLL(_SO_PATH)
    except OSError:
        return None
    if not hasattr(lib, "axon_start_nrt_profile"):
        return None
    lib.axon_start_nrt_profile.argtypes = [ctypes.POINTER(ctypes.c_int64), ctypes.c_size_t]
    lib.axon_start_nrt_profile.restype = ctypes.c_int64
    lib.axon_stop_nrt_profile.argtypes = [ctypes.c_char_p]
    lib.axon_stop_nrt_profile.restype = ctypes.c_int64

    @contextlib.contextmanager
    def _hook(output_dir, device_ids):
        import jax

        jax.devices()
        if device_ids:
            ids = (ctypes.c_int64 * len(device_ids))(*device_ids)
            rc = lib.axon_start_nrt_profile(ids, len(device_ids))
        else:
            rc = lib.axon_start_nrt_profile(None, 0)
        if rc != 0:
            raise RuntimeError(f"axon_start_nrt_profile rc={rc}")
        try:
            yield
        finally:
            n = lib.axon_stop_nrt_profile(str(output_dir).encode())
            if n <= 0:
                print(f"ntff profile: rc={n} (no files?) dir={output_dir}", file=sys.stderr)

    return _hook


def _install_ntff_shim():
    if "antenv.axon_hooks" in sys.modules:
        return
    mod = types.ModuleType("antenv.axon_hooks")
    _hook = _make_ntff_hook()
    mod.get_axon_ntff_profile_hook = lambda: _hook
    mod.set_axon_ntff_profile_hook = lambda h: None
    sys.modules["antenv.axon_hooks"] = mod
    try:
        import antenv

        antenv.axon_hooks = mod
    except ImportError:
        pass


_install_ntff_shim()

# ---------------------------------------------------------------------------

PLAN = os.environ.get("KERNEL_PLAN", "Z")
USE_GP = os.environ.get("KERNEL_GP", "0") == "1"
# Width of the per-band mask-add slice offloaded to GPSIMD (0 = all on DVE)
GPW = int(os.environ.get("KERNEL_GPW", "0"))
# Number of B-type (ACT-heavy) bands out of nb=32.  A-band: DVE TS+TT+TT,
# ACT Relu+accum.  B-band: ACT Prelu+Exp+accum, DVE mask-TT only.
# Measured per-band costs: A = 6.29us DVE / 3.97us ACT; B = 2.68 / 7.66.
# Balance lands at ~10/32.  B bands live in [2, nb-6]: none at the start
# (ramp: prelu would queue behind the preamble exps) and none at the end
# (drain: the final A-run lets ACT catch up before the last band's chain).
NB_B = int(os.environ.get("KERNEL_NB", "10"))


def build_nc(s=S, plan=None):
    """Build + compile the per-core Bass program (same program on all cores)."""
    plan = plan or PLAN
    nb = s // 128     # number of j-bands
    nch = s // 512    # number of 512-wide i-chunks

    nc = bacc.Bacc("TRN2", target_bir_lowering=False, debug=False, enable_asserts=False)

    xt = nc.dram_tensor("xt", [FIN, s], DT.float16, kind="ExternalInput").ap()
    w = nc.dram_tensor("w", [FIN, FOUT], DT.float16, kind="ExternalInput").ap()
    wt = nc.dram_tensor("wt", [FOUT, FIN], DT.float16, kind="ExternalInput").ap()
    a12 = nc.dram_tensor("a12", [FOUT, 2], DT.float16, kind="ExternalInput").ap()
    adjt = nc.dram_tensor("adjt", [s, s], DT.float16, kind="ExternalInput").ap()
    out = nc.dram_tensor("out", [FOUT, s], DT.float16, kind="ExternalOutput").ap()

    with tile.TileContext(nc) as tc, ExitStack() as ctx:
        _body(ctx, tc, nc, xt, w, wt, a12, adjt, out, s, nb, nch, plan)

    if os.environ.get("KERNEL_LDW1", "1") == "1":
        # Mark matmuls whose stationary operand AP repeats the immediately
        # preceding matmul's as non-self-loading (PE keeps the loaded array).
        n_marked = 0
        for blk in nc.m.functions[0].blocks:
            prev_w = None
            for inst in blk.instructions:
                if type(inst).__name__ != "InstMatmult":
                    continue
                wkey = repr(inst.ins[1])
                if prev_w == wkey:
                    inst.ldweights = False
                    n_marked += 1
                prev_w = wkey
        print(f"KERNEL_LDW1: marked {n_marked} matmuls non-self-loading")

    nc.compile()
    return nc


def _body(ctx, tc, nc, xt, w, wt, a12, adjt, out, s, nb, nch, plan):
    f32, f16 = DT.float32, DT.float16

    if NB_B > 1:
        bset = {2 + round(k * (nb - 8) / (NB_B - 1)) for k in range(NB_B)}
    else:
        bset = {nb // 2} if NB_B else set()

    def is_b(b):
        """True if band b runs the ACT-heavy path."""
        return b in bset

    # ---------------- persistent intermediates (live through main loop) ----
    cpool = ctx.enter_context(tc.tile_pool(name="const", bufs=1))
    r1b_sb = cpool.tile([128, s], f16, tag="r1b")      # exp(0.8 f1[i]) bcast (A bands)
    e1sb_sb = cpool.tile([128, s], f16, tag="e1sb")    # exp(0.2 f1[i]) bcast (A bands)
    f1b_sb = cpool.tile([128, s], f16, tag="f1b")      # f1[i] bcast (B bands)
    hf_sb = cpool.tile([128, nb * (FOUT + 1)], f32, tag="hf")  # [h|f2] per band
    eu_sb = cpool.tile([128, nb], f32, tag="eu")       # exp(f2), band b in col b
    ev_sb = cpool.tile([128, nb], f32, tag="ev")       # exp(0.2 f2)

    def h_col(b):
        return hf_sb[:, b * (FOUT + 1) : b * (FOUT + 1) + FOUT]

    def f2_col(b):
        return hf_sb[:, b * (FOUT + 1) + FOUT : (b + 1) * (FOUT + 1)]

    # ---------------- preamble (scoped pools, freed before main loop) ------
    with tc.tile_pool(name="pre_sb", bufs=1) as tpool:
        # weights first: tiny transfers that gate the wa matmul
        w65_sb = tpool.tile([FIN, FOUT + 1], f16, tag="w65")
        nc.sync.dma_start(w65_sb[:, 0:FOUT], w[:])
        wt_sb = tpool.tile([FOUT, FIN], f16, tag="wt")
        nc.sync.dma_start(wt_sb[:], wt[:])
        a12_sb = tpool.tile([FOUT, 2], f16, tag="a12")
        nc.sync.dma_start(a12_sb[:], a12[:])
        xt_sb = tpool.tile([FIN, s], f16, tag="xt")
        # split across partition-groups x column-halves: a single 1MB DMA
        # moves ~128 x 8KB packets serially on ONE hw queue (~26.5GB/s,
        # ~40us latency); 8 slices land in parallel on separate queues.
        for pg in range(4):
            for ch in range(2):
                nc.sync.dma_start(
                    xt_sb[32 * pg : 32 * (pg + 1), bass.ts(ch, s // 2)],
                    xt[32 * pg : 32 * (pg + 1), bass.ts(ch, s // 2)],
                )
        wa_sb = tpool.tile([FIN, 2], f16, tag="wa")    # [wa1 | wa2]

        # wa = W @ [a1 | a2]  (contract over FOUT)
        with tc.tile_pool(name="pre_wa", bufs=1, space="PSUM") as pwa:
            wa_ps = pwa.tile([FIN, 2], f32, tag="wa")
            nc.tensor.matmul(wa_ps[:], lhsT=wt_sb[:], rhs=a12_sb[:], start=True, stop=True)
            nc.vector.tensor_copy(wa_sb[:], wa_ps[:])
            nc.vector.tensor_copy(w65_sb[:, FOUT : FOUT + 1], wa_ps[:, 1:2])

        # zb[p, i] = f1[i] for all partitions p, computed directly via a
        # stride-0 broadcast stationary (wa1 replicated across 128 array
        # columns).  Exp(0.8 z)/Exp(0.2 z) feed the A bands straight out of
        # PSUM (no DVE cost); the raw f16 copy of z feeds the B bands.
        wa1rep = wa_sb[:, 0:1].broadcast_to([FIN, 128])
        with tc.tile_pool(name="pre_bc", bufs=2, space="PSUM") as pbcp:
            for c in range(s // 1024):
                lo = c * 1024
                pbc = pbcp.tile([128, 1024], f32, tag="bc")
                for k in range(2):
                    nc.tensor.matmul(pbc[:, bass.ts(k, 512)], lhsT=wa1rep,
                                     rhs=xt_sb[:, lo + k * 512 : lo + (k + 1) * 512],
                                     start=True, stop=True)
                nc.scalar.activation(r1b_sb[:, bass.ts(c, 1024)], pbc[:], AF.Exp, scale=0.8)
                nc.scalar.activation(e1sb_sb[:, bass.ts(c, 1024)], pbc[:], AF.Exp, scale=0.2)
                nc.vector.tensor_copy(f1b_sb[:, bass.ts(c, 1024)], pbc[:])

        # [h_band | f2_band] = xt_band^T @ [W | wa2]  (one matmul + one copy
        # per band into the combined hf tile; f2 stays at stride FOUT+1)
        with tc.tile_pool(name="pre_h", bufs=3, space="PSUM") as phf:
            for b in range(nb):
                ph = phf.tile([128, FOUT + 1], f32, tag="hf")
                nc.tensor.matmul(ph[:], lhsT=xt_sb[:, bass.ts(b, 128)], rhs=w65_sb[:], start=True, stop=True)
                nc.vector.tensor_copy(hf_sb[:, bass.ts(b, FOUT + 1)], ph[:])

        # exp of f2 cols (strided reads of the hf tile; tiny ACT ops)
        f2_strided = hf_sb[:, FOUT :: FOUT + 1]
        nc.scalar.activation(eu_sb[:], f2_strided, AF.Exp)
        nc.scalar.activation(ev_sb[:], f2_strided, AF.Exp, scale=0.2)

    # ---------------- main loop over j-bands ----------------
    upool = ctx.enter_context(tc.tile_pool(name="umax", bufs=2))
    ppool = ctx.enter_context(tc.tile_pool(name="pmat", bufs=4))
    scrpool = ctx.enter_context(tc.tile_pool(name="scr", bufs=4))
    spool = ctx.enter_context(tc.tile_pool(name="svec", bufs=12))
    hppool = ctx.enter_context(tc.tile_pool(name="hp", bufs=3))
    mpool = ctx.enter_context(tc.tile_pool(name="mask", bufs=4))
    qpool = ctx.enter_context(tc.tile_pool(name="qtmp", bufs=2))
    wpool = ctx.enter_context(tc.tile_pool(name="wlr", bufs=3))

    mainpsum = ctx.enter_context(tc.tile_pool(name="out_psum", bufs=1, space="PSUM"))
    psum_out = mainpsum.tile([FOUT, s], f32, tag="out")

    def finish_band(b, p_t, s_t, elu_emit=None):
        """Reciprocal + hp scaling + accumulating matmuls for band b.

        Deferred one iteration so the DVE FIFO's reciprocal never waits on
        the ACT passes of the same band (head-of-line stall)."""
        hp_t = hppool.tile([128, FOUT], f16, tag="hp")
        rs_t = spool.tile([128, 1], f32, tag="rs")
        nc.vector.reciprocal(rs_t[:], s_t[:])
        # hp = h * (1/s): per-partition scalar mult on DVE (cheap TS) keeps
        # the ACT queue free for the big relu/prelu/exp passes.
        nc.vector.tensor_scalar(out=hp_t[:], in0=h_col(b), scalar1=rs_t[:],
                                scalar2=None, op0=ALU.mult)
        for c in range(nch):
            nc.tensor.matmul(
                psum_out[:, bass.ts(c, 512)], lhsT=hp_t[:], rhs=p_t[:, bass.ts(c, 512)],
                start=(b == 0), stop=(b == nb - 1),
            )
            if elu_emit is not None:
                elu_emit(c)

    # w tiles (Prelu of f1+f2) for B bands, computed one band ahead
    w_tiles = {}

    def emit_prelu(b):
        w_t = wpool.tile([128, s], f16, tag="w")
        nc.scalar.activation(w_t[:], f1b_sb[:], AF.Prelu, bias=f2_col(b), alpha=0.2)
        w_tiles[b] = w_t

    # prefetch the first few band masks so band 0 never waits on DMA
    def mask_dma(b, nsplit=1):
        # nsplit>1 halves/quarters the on-queue latency (1MB = ~40us on one
        # hw queue) at a cost of ~0.6us dispatch each — only worth it for
        # the ramp-gating first masks.
        m_t = mpool.tile([128, s], f16, tag="m")
        pw = 128 // nsplit
        for pg in range(nsplit):
            nc.sync.dma_start(m_t[pw * pg : pw * (pg + 1), :],
                              adjt[b * 128 + pw * pg : b * 128 + pw * (pg + 1), :])
        return m_t

    m_tiles = {}
    for b in range(min(3, nb)):
        m_tiles[b] = mask_dma(b, nsplit=4)

    if is_b(0):
        emit_prelu(0)
    pending = None
    for b in range(nb):
        if b in m_tiles:
            m_t = m_tiles.pop(b)
        else:
            m_t = mask_dma(b)
        p_t = scrpool.tile([128, s], f16, tag="p")
        s_t = spool.tile([128, 1], f32, tag="s")

        if b + 1 < nb and is_b(b + 1):
            emit_prelu(b + 1)  # ACT fills while DVE works on band b

        if not is_b(b):
            # ---- A band (DVE-heavy): u'=max(R1b*eu,ev); q=u'*E1sb; +mask;
            #      ACT Relu zeroes masked entries and row-sums.
            u_t = upool.tile([128, s], f16, tag="u")
            nc.vector.tensor_scalar(
                out=u_t[:], in0=r1b_sb[:], scalar1=eu_sb[:, b : b + 1],
                scalar2=ev_sb[:, b : b + 1], op0=ALU.mult, op1=ALU.max,
            )
            q_t = qpool.tile([128, s], f16, tag="q")
            nc.vector.tensor_tensor(out=q_t[:], in0=u_t[:], in1=e1sb_sb[:], op=ALU.mult)
            if pending is not None:
                finish_band(*pending)
                pending = None
            pb_t = ppool.tile([128, s], f16, tag="pb")
            nc.vector.tensor_tensor(out=pb_t[:], in0=q_t[:], in1=m_t[:], op=ALU.add)
            nc.scalar.activation(p_t[:], pb_t[:], AF.Relu, accum_out=s_t[:])
        else:
            # ---- B band (ACT-heavy): pb = prelu(f1+f2) + mask; p = exp(pb)
            w_t = w_tiles.pop(b)
            pb_t = ppool.tile([128, s], f16, tag="pb")
            nc.vector.tensor_tensor(out=pb_t[:], in0=w_t[:], in1=m_t[:], op=ALU.add)
            if pending is not None:
                finish_band(*pending)
                pending = None
            nc.scalar.activation(p_t[:], pb_t[:], AF.Exp, accum_out=s_t[:])

        pending = (b, p_t, s_t)

    # ---------------- ELU + writeout, pipelined with the last band's stop-
    # matmuls: chunk c's ELU is emitted right after its final accumulation.
    fpool = ctx.enter_context(tc.tile_pool(name="fin", bufs=2))
    ew = s // nch  # 512

    def elu_emit(c):
        sl = bass.ts(c, ew)
        r_t = fpool.tile([FOUT, ew], f32, tag="relu")
        nc.scalar.activation(r_t[:], psum_out[:, sl], AF.Relu)
        mn_t = fpool.tile([FOUT, ew], f32, tag="min")
        nc.vector.tensor_scalar_min(out=mn_t[:], in0=psum_out[:, sl], scalar1=0.0)
        e_t = fpool.tile([FOUT, ew], f32, tag="exp")
        nc.scalar.activation(e_t[:], mn_t[:], AF.Exp)
        f_t = fpool.tile([FOUT, ew], f16, tag="fin")
        # f = (e - 1) + r   (f16 out halves the writeback; host upcasts)
        nc.vector.scalar_tensor_tensor(out=f_t[:], in0=e_t[:], scalar=-1.0, in1=r_t[:], op0=ALU.add, op1=ALU.add)
        nc.sync.dma_start(out[:, sl], f_t[:])

    finish_band(*pending, elu_emit=elu_emit)


_NC_CACHE = {}


def _get_nc(s=S, plan=None):
    key = (s, plan or PLAN)
    if key not in _NC_CACHE:
        _NC_CACHE[key] = build_nc(s, plan)
    return _NC_CACHE[key]


def kernel(input_seq, adj, W, a_1, a_2):
    """Full-input entry point: shards by head across 8 cores, returns [S, H*FOUT]."""
    global LAST_RESULTS
    X = np.asarray(input_seq)[0]          # [S, FIN] f32
    adjm = np.asarray(adj)[0]             # [S, S] int32
    Wn = np.asarray(W)                    # [H, FIN, FOUT]
    a1n = np.asarray(a_1)                 # [H, FOUT, 1]
    a2n = np.asarray(a_2)                 # [H, FOUT, 1]

    s = X.shape[0]
    xt = np.ascontiguousarray(X.T, dtype=np.float16)
    # mask encoded as {0, -BIG}: p = relu(p + mask') zeroes masked-out entries
    adjt = np.where(np.ascontiguousarray(adjm.T) != 0, np.float16(0.0), np.float16(-60000.0))

    nc = _get_nc(s)
    in_maps = [
        {
            "xt": xt,
            "w": np.ascontiguousarray(Wn[h], dtype=np.float16),
            "wt": np.ascontiguousarray(Wn[h].T, dtype=np.float16),
            "a12": np.ascontiguousarray(
                np.concatenate([a1n[h], a2n[h]], axis=1), dtype=np.float16
            ),
            "adjt": adjt,
        }
        for h in range(H)
    ]
    res = bass_utils.run_bass_kernel_spmd(nc, in_maps, core_ids=list(range(H)))
    LAST_RESULTS = res

    outf = np.empty((s, H * FOUT), dtype=np.float32)
    for h in range(H):
        outf[:, h * FOUT : (h + 1) * FOUT] = res.results[h]["out"].T
    return outf



# revision 21
# speedup vs baseline: 1.0791x; 1.0252x over previous
"""GAT multi-head attention layer (nn_GATMutiHeadAttLayer) on 8 Trainium2 cores.

Head-sharded: core h computes head h entirely (no collectives).

Math (per head):
  h = X @ W                       [S, FOUT]
  f1 = h @ a1, f2 = h @ a2        [S]
  e[i,j] = lrelu(f1[i] + f2[j], 0.2), masked by adj[i,j]
  attn = softmax(e, axis=i)  (denominator s[j] = sum_i)
  out = attn @ h, concat heads, ELU.

Device formulation (transposed, j on partitions; fp16 tiles).  All per-band
elementwise ops are chosen for the DVE's fast uop modes (TS = 2 results/cyc
with per-partition scalar pair; TT = 2 results/cyc; STT/TTR are 1x and
avoided):
  exp(lrelu(z)) = max(exp(z), exp(0.2 z)),  z = f1[i] + f2[j]
  u'[j,i] = max(R1b[i] * eu[j], ev[j])          (TS: mult, max)
     R1b = exp(0.8 f1) bcast, eu = exp(f2), ev = exp(0.2 f2)
  q[j,i]  = u' * E1sb[i]                        (TT: mult)
     E1sb = exp(0.2 f1) bcast;  q = exp(lrelu(z)) unmasked, >= 0
  pb = q + mask'  (mask' in {0, -BIG} fp16, DMA'd)   (TT: add)
  p = relu(pb), s[j] = sum_i p   (one ACT pass: zeroes masked entries AND
                                  row-sums via the fused accumulator)
  hp = h[band] * (1/s)  (DVE reciprocal + ACT copy-with-scale; deferred one
                         band so the DVE FIFO never stalls on the ACT pass)
  out^T[o,i] = sum_j hp[j,o] * p[j,i]   (PE, PSUM-accumulated over bands)
  final: ELU(out^T) -> DRAM; host transposes/concats heads.

Preamble (PE fp16):
  wa = W @ [a1|a2] via W^T; zb[p,i] = f1[i] directly via a stride-0
  broadcast stationary (wa1 replicated); R1b/E1sb = Exp(0.8/0.2 * zb)
  straight out of PSUM.  [h_band | f2_band] = xt_band^T @ [W | wa2]
  (one N=65 matmul per band);  eu = exp(f2), ev = exp(0.2 f2).

Host prep: X^T, W, W^T, [a1|a2] cast fp16; adj^T encoded {0, -60000} fp16.
All model compute (matmuls, exp, masking, softmax, ELU) runs on device.

Rejected on measurement: STT/TTR formulations (1x uop, 4.4us/band), SWDGE
accumulate-DMA masking (runtime crash), GPSIMD elementwise offload (shared
SBUF port degrades DVE fast modes 2-4x), GPSIMD normalize_recip (2us/call
on the critical chain).
"""

import contextlib
import ctypes
import os
import sys
import types
from contextlib import ExitStack

import numpy as np

import concourse.bass as bass
import concourse.tile as tile
from concourse import bacc, mybir
from concourse import bass_utils

AF = mybir.ActivationFunctionType
ALU = mybir.AluOpType
DT = mybir.dt

S = 4096
FIN = 128
FOUT = 64
H = 8
ALPHA = 0.2

LAST_RESULTS = None  # BassKernelResults of the most recent run (for test harness)

# ---------------------------------------------------------------------------
# NTFF profile hook shim: antenv.axon_hooks is absent in this container; the
# trace=True path of run_bass_kernel_spmd imports it. Recreate it via ctypes
# against libaxon_pjrt.so (same as trn_agent_boot does).
_SO_PATH = "/opt/axon/libaxon_pjrt.so"


def _make_ntff_hook():
    try:
        lib = ctypes.CDLL(_SO_PATH)
    except OSError:
        return None
    if not hasattr(lib, "axon_start_nrt_profile"):
        return None
    lib.axon_start_nrt_profile.argtypes = [ctypes.POINTER(ctypes.c_int64), ctypes.c_size_t]
    lib.axon_start_nrt_profile.restype = ctypes.c_int64
    lib.axon_stop_nrt_profile.argtypes = [ctypes.c_char_p]
    lib.axon_stop_nrt_profile.restype = ctypes.c_int64

    @contextlib.contextmanager
    def _hook(output_dir, device_ids):
        import jax

        jax.devices()
        if device_ids:
            ids = (ctypes.c_int64 * len(device_ids))(*device_ids)
            rc = lib.axon_start_nrt_profile(ids, len(device_ids))
        else:
            rc = lib.axon_start_nrt_profile(None, 0)
        if rc != 0:
            raise RuntimeError(f"axon_start_nrt_profile rc={rc}")
        try:
            yield
        finally:
            n = lib.axon_stop_nrt_profile(str(output_dir).encode())
            if n <= 0:
                print(f"ntff profile: rc={n} (no files?) dir={output_dir}", file=sys.stderr)

    return _hook


def _install_ntff_shim():
    if "antenv.axon_hooks" in sys.modules:
        return
    mod = types.ModuleType("antenv.axon_hooks")
    _hook = _make_ntff_hook()
    mod.get_axon_ntff_profile_hook = lambda: _hook
    mod.set_axon_ntff_profile_hook = lambda h: None
    sys.modules["antenv.axon_hooks"] = mod
    try:
        import antenv

        antenv.axon_hooks = mod
    except ImportError:
        pass


_install_ntff_shim()

# ---------------------------------------------------------------------------

PLAN = os.environ.get("KERNEL_PLAN", "Z")
USE_GP = os.environ.get("KERNEL_GP", "0") == "1"
# Width of the per-band mask-add slice offloaded to GPSIMD (0 = all on DVE)
GPW = int(os.environ.get("KERNEL_GPW", "0"))
# Number of B-type (ACT-heavy) bands out of nb=32.  A-band: DVE TS+TT+TT,
# ACT Relu+accum.  B-band: ACT Prelu+Exp+accum, DVE mask-TT only.
# Measured per-band costs: A = 6.29us DVE / 3.97us ACT; B = 2.68 / 7.66.
# Balance lands at ~10/32.  B bands live in [2, nb-6]: none at the start
# (ramp: prelu would queue behind the preamble exps) and none at the end
# (drain: the final A-run lets ACT catch up before the last band's chain).
NB_B = int(os.environ.get("KERNEL_NB", "10"))


def build_nc(s=S, plan=None):
    """Build + compile the per-core Bass program (same program on all cores)."""
    plan = plan or PLAN
    nb = s // 128     # number of j-bands
    nch = s // 512    # number of 512-wide i-chunks

    nc = bacc.Bacc("TRN2", target_bir_lowering=False, debug=False, enable_asserts=False)

    xt = nc.dram_tensor("xt", [FIN, s], DT.float16, kind="ExternalInput").ap()
    w = nc.dram_tensor("w", [FIN, FOUT], DT.float16, kind="ExternalInput").ap()
    wt = nc.dram_tensor("wt", [FOUT, FIN], DT.float16, kind="ExternalInput").ap()
    a12 = nc.dram_tensor("a12", [FOUT, 2], DT.float16, kind="ExternalInput").ap()
    adjt = nc.dram_tensor("adjt", [s, s], DT.float16, kind="ExternalInput").ap()
    out = nc.dram_tensor("out", [FOUT, s], DT.float16, kind="ExternalOutput").ap()

    with tile.TileContext(nc) as tc, ExitStack() as ctx:
        _body(ctx, tc, nc, xt, w, wt, a12, adjt, out, s, nb, nch, plan)

    if os.environ.get("KERNEL_LDW1", "1") == "1":
        # Mark matmuls whose stationary operand AP repeats the immediately
        # preceding matmul's as non-self-loading (PE keeps the loaded array).
        n_marked = 0
        for blk in nc.m.functions[0].blocks:
            prev_w = None
            for inst in blk.instructions:
                if type(inst).__name__ != "InstMatmult":
                    continue
                wkey = repr(inst.ins[1])
                if prev_w == wkey:
                    inst.ldweights = False
                    n_marked += 1
                prev_w = wkey
        print(f"KERNEL_LDW1: marked {n_marked} matmuls non-self-loading")

    nc.compile()
    return nc


def _body(ctx, tc, nc, xt, w, wt, a12, adjt, out, s, nb, nch, plan):
    f32, f16 = DT.float32, DT.float16

    if NB_B > 1:
        bset = {2 + round(k * (nb - 8) / (NB_B - 1)) for k in range(NB_B)}
    else:
        bset = {nb // 2} if NB_B else set()

    def is_b(b):
        """True if band b runs the ACT-heavy path."""
        return b in bset

    # ---------------- persistent intermediates (live through main loop) ----
    cpool = ctx.enter_context(tc.tile_pool(name="const", bufs=1))
    r1b_sb = cpool.tile([128, s], f16, tag="r1b")      # exp(0.8 f1[i]) bcast (A bands)
    e1sb_sb = cpool.tile([128, s], f16, tag="e1sb")    # exp(0.2 f1[i]) bcast (A bands)
    f1b_sb = cpool.tile([128, s], f16, tag="f1b")      # f1[i] bcast (B bands)
    hf_sb = cpool.tile([128, nb * (FOUT + 1)], f32, tag="hf")  # [h|f2] per band
    eu_sb = cpool.tile([128, nb], f32, tag="eu")       # exp(f2), band b in col b
    ev_sb = cpool.tile([128, nb], f32, tag="ev")       # exp(0.2 f2)

    def h_col(b):
        return hf_sb[:, b * (FOUT + 1) : b * (FOUT + 1) + FOUT]

    def f2_col(b):
        return hf_sb[:, b * (FOUT + 1) + FOUT : (b + 1) * (FOUT + 1)]

    # ---------------- preamble (scoped pools, freed before main loop) ------
    with tc.tile_pool(name="pre_sb", bufs=1) as tpool:
        # weights first: tiny transfers that gate the wa matmul
        w65_sb = tpool.tile([FIN, FOUT + 1], f16, tag="w65")
        nc.sync.dma_start(w65_sb[:, 0:FOUT], w[:])
        wt_sb = tpool.tile([FOUT, FIN], f16, tag="wt")
        nc.sync.dma_start(wt_sb[:], wt[:])
        a12_sb = tpool.tile([FOUT, 2], f16, tag="a12")
        nc.sync.dma_start(a12_sb[:], a12[:])
        xt_sb = tpool.tile([FIN, s], f16, tag="xt")
        # split across partition-groups x column-halves: a single 1MB DMA
        # moves ~128 x 8KB packets serially on ONE hw queue (~26.5GB/s,
        # ~40us latency); 8 slices land in parallel on separate queues.
        for pg in range(4):
            for ch in range(2):
                eng = nc.scalar if ch == 0 else nc.sync
                eng.dma_start(
                    xt_sb[32 * pg : 32 * (pg + 1), bass.ts(ch, s // 2)],
                    xt[32 * pg : 32 * (pg + 1), bass.ts(ch, s // 2)],
                )
        wa_sb = tpool.tile([FIN, 2], f16, tag="wa")    # [wa1 | wa2]

        # wa = W @ [a1 | a2]  (contract over FOUT)
        with tc.tile_pool(name="pre_wa", bufs=1, space="PSUM") as pwa:
            wa_ps = pwa.tile([FIN, 2], f32, tag="wa")
            nc.tensor.matmul(wa_ps[:], lhsT=wt_sb[:], rhs=a12_sb[:], start=True, stop=True)
            nc.vector.tensor_copy(wa_sb[:], wa_ps[:])
            nc.vector.tensor_copy(w65_sb[:, FOUT : FOUT + 1], wa_ps[:, 1:2])

        # zb[p, i] = f1[i] for all partitions p, computed directly via a
        # stride-0 broadcast stationary (wa1 replicated across 128 array
        # columns).  Exp(0.8 z)/Exp(0.2 z) feed the A bands straight out of
        # PSUM (no DVE cost); the raw f16 copy of z feeds the B bands.
        # bands 0/1 are processed column-chunked right after each bcast
        # chunk lands; their h/f2/eu/ev must be ready first (tiny ops)
        with tc.tile_pool(name="pre_h0", bufs=2, space="PSUM") as ph0:
            for b in range(2):
                ph = ph0.tile([128, FOUT + 1], f32, tag="hf0")
                nc.tensor.matmul(ph[:], lhsT=xt_sb[:, bass.ts(b, 128)], rhs=w65_sb[:], start=True, stop=True)
                nc.vector.tensor_copy(hf_sb[:, bass.ts(b, FOUT + 1)], ph[:])
                nc.scalar.activation(eu_sb[:, b : b + 1], f2_col(b), AF.Exp)
                nc.scalar.activation(ev_sb[:, b : b + 1], f2_col(b), AF.Exp, scale=0.2)

        wa1rep = wa_sb[:, 0:1].broadcast_to([FIN, 128])
        with tc.tile_pool(name="pre_bc", bufs=2, space="PSUM") as pbcp:
            for c in range(s // 1024):
                lo = c * 1024
                pbc = pbcp.tile([128, 1024], f32, tag="bc")
                for k in range(2):
                    nc.tensor.matmul(pbc[:, bass.ts(k, 512)], lhsT=wa1rep,
                                     rhs=xt_sb[:, lo + k * 512 : lo + (k + 1) * 512],
                                     start=True, stop=True)
                nc.scalar.activation(r1b_sb[:, bass.ts(c, 1024)], pbc[:], AF.Exp, scale=0.8)
                nc.scalar.activation(e1sb_sb[:, bass.ts(c, 1024)], pbc[:], AF.Exp, scale=0.2)
                nc.vector.tensor_copy(f1b_sb[:, bass.ts(c, 1024)], pbc[:])

        # [h_band | f2_band] = xt_band^T @ [W | wa2]  (one matmul + one copy
        # per band into the combined hf tile; f2 stays at stride FOUT+1)
        with tc.tile_pool(name="pre_h", bufs=3, space="PSUM") as phf:
            for b in range(2, nb):
                ph = phf.tile([128, FOUT + 1], f32, tag="hf")
                nc.tensor.matmul(ph[:], lhsT=xt_sb[:, bass.ts(b, 128)], rhs=w65_sb[:], start=True, stop=True)
                nc.vector.tensor_copy(hf_sb[:, bass.ts(b, FOUT + 1)], ph[:])

        # exp of f2 cols for bands 2+ (strided reads of hf; tiny ACT ops)
        off = 2 * (FOUT + 1) + FOUT
        f2_strided = hf_sb[:, off :: FOUT + 1]
        nc.scalar.activation(eu_sb[:, 2:nb], f2_strided, AF.Exp)
        nc.scalar.activation(ev_sb[:, 2:nb], f2_strided, AF.Exp, scale=0.2)

    # ---------------- main loop over j-bands ----------------
    upool = ctx.enter_context(tc.tile_pool(name="umax", bufs=2))
    ppool = ctx.enter_context(tc.tile_pool(name="pmat", bufs=4))
    scrpool = ctx.enter_context(tc.tile_pool(name="scr", bufs=4))
    spool = ctx.enter_context(tc.tile_pool(name="svec", bufs=12))
    hppool = ctx.enter_context(tc.tile_pool(name="hp", bufs=3))
    mpool = ctx.enter_context(tc.tile_pool(name="mask", bufs=4))
    qpool = ctx.enter_context(tc.tile_pool(name="qtmp", bufs=2))
    wpool = ctx.enter_context(tc.tile_pool(name="wlr", bufs=3))

    mainpsum = ctx.enter_context(tc.tile_pool(name="out_psum", bufs=1, space="PSUM"))
    psum_out = mainpsum.tile([FOUT, s], f32, tag="out")

    def finish_band(b, p_t, s_t, elu_emit=None):
        """Reciprocal + hp scaling + accumulating matmuls for band b.

        Deferred one iteration so the DVE FIFO's reciprocal never waits on
        the ACT passes of the same band (head-of-line stall)."""
        hp_t = hppool.tile([128, FOUT], f16, tag="hp")
        rs_t = spool.tile([128, 1], f32, tag="rs")
        nc.vector.reciprocal(rs_t[:], s_t[:])
        # hp = h * (1/s): per-partition scalar mult on DVE (cheap TS) keeps
        # the ACT queue free for the big relu/prelu/exp passes.
        nc.vector.tensor_scalar(out=hp_t[:], in0=h_col(b), scalar1=rs_t[:],
                                scalar2=None, op0=ALU.mult)
        for c in range(nch):
            nc.tensor.matmul(
                psum_out[:, bass.ts(c, 512)], lhsT=hp_t[:], rhs=p_t[:, bass.ts(c, 512)],
                start=(b == 0), stop=(b == nb - 1),
            )
            if elu_emit is not None:
                elu_emit(c)

    # w tiles (Prelu of f1+f2) for B bands, computed one band ahead
    w_tiles = {}

    def emit_prelu(b):
        w_t = wpool.tile([128, s], f16, tag="w")
        nc.scalar.activation(w_t[:], f1b_sb[:], AF.Prelu, bias=f2_col(b), alpha=0.2)
        w_tiles[b] = w_t

    # prefetch the first few band masks so band 0 never waits on DMA
    def mask_dma(b, nsplit=1):
        # nsplit>1 halves/quarters the on-queue latency (1MB = ~40us on one
        # hw queue) at a cost of ~0.6us dispatch each — only worth it for
        # the ramp-gating first masks.
        m_t = mpool.tile([128, s], f16, tag="m")
        pw = 128 // nsplit
        for pg in range(nsplit):
            nc.sync.dma_start(m_t[pw * pg : pw * (pg + 1), :],
                              adjt[b * 128 + pw * pg : b * 128 + pw * (pg + 1), :])
        return m_t

    m_tiles = {}
    for b in range(min(3, nb)):
        m_tiles[b] = mask_dma(b, nsplit=4)

    if is_b(0):
        emit_prelu(0)
    pending = None
    for b in range(nb):
        if b in m_tiles:
            m_t = m_tiles.pop(b)
        else:
            m_t = mask_dma(b)
        p_t = scrpool.tile([128, s], f16, tag="p")
        s_t = spool.tile([128, 1], f32, tag="s")

        if b + 1 < nb and is_b(b + 1):
            emit_prelu(b + 1)  # ACT fills while DVE works on band b

        if not is_b(b):
            # ---- A band (DVE-heavy): u'=max(R1b*eu,ev); q=u'*E1sb; +mask;
            #      ACT Relu zeroes masked entries and row-sums.
            # Bands 0/1 run column-chunked so each chunk starts as soon as
            # its slice of the bcast preamble lands (ramp overlap).
            nchk = 4 if b < 2 else 1
            cw = s // nchk
            u_t = upool.tile([128, s], f16, tag="u")
            q_t = qpool.tile([128, s], f16, tag="q")
            pb_t = ppool.tile([128, s], f16, tag="pb")
            for c in range(nchk):
                sl = bass.ts(c, cw)
                nc.vector.tensor_scalar(
                    out=u_t[:, sl], in0=r1b_sb[:, sl], scalar1=eu_sb[:, b : b + 1],
                    scalar2=ev_sb[:, b : b + 1], op0=ALU.mult, op1=ALU.max,
                )
                nc.vector.tensor_tensor(out=q_t[:, sl], in0=u_t[:, sl], in1=e1sb_sb[:, sl], op=ALU.mult)
                nc.vector.tensor_tensor(out=pb_t[:, sl], in0=q_t[:, sl], in1=m_t[:, sl], op=ALU.add)
                if pending is not None:
                    finish_band(*pending)
                    pending = None
            nc.scalar.activation(p_t[:], pb_t[:], AF.Relu, accum_out=s_t[:])
        else:
            # ---- B band (ACT-heavy): pb = prelu(f1+f2) + mask; p = exp(pb)
            w_t = w_tiles.pop(b)
            pb_t = ppool.tile([128, s], f16, tag="pb")
            nc.vector.tensor_tensor(out=pb_t[:], in0=w_t[:], in1=m_t[:], op=ALU.add)
            if pending is not None:
                finish_band(*pending)
                pending = None
            nc.scalar.activation(p_t[:], pb_t[:], AF.Exp, accum_out=s_t[:])

        pending = (b, p_t, s_t)

    # ---------------- ELU + writeout, pipelined with the last band's stop-
    # matmuls: chunk c's ELU is emitted right after its final accumulation.
    fpool = ctx.enter_context(tc.tile_pool(name="fin", bufs=2))
    ew = s // nch  # 512

    def elu_emit(c):
        sl = bass.ts(c, ew)
        r_t = fpool.tile([FOUT, ew], f32, tag="relu")
        nc.scalar.activation(r_t[:], psum_out[:, sl], AF.Relu)
        mn_t = fpool.tile([FOUT, ew], f32, tag="min")
        nc.vector.tensor_scalar_min(out=mn_t[:], in0=psum_out[:, sl], scalar1=0.0)
        e_t = fpool.tile([FOUT, ew], f32, tag="exp")
        nc.scalar.activation(e_t[:], mn_t[:], AF.Exp)
        f_t = fpool.tile([FOUT, ew], f16, tag="fin")
        # f = (e - 1) + r   (f16 out halves the writeback; host upcasts)
        nc.vector.scalar_tensor_tensor(out=f_t[:], in0=e_t[:], scalar=-1.0, in1=r_t[:], op0=ALU.add, op1=ALU.add)
        nc.sync.dma_start(out[:, sl], f_t[:])

    finish_band(*pending, elu_emit=elu_emit)


_NC_CACHE = {}


def _get_nc(s=S, plan=None):
    key = (s, plan or PLAN)
    if key not in _NC_CACHE:
        _NC_CACHE[key] = build_nc(s, plan)
    return _NC_CACHE[key]


def kernel(input_seq, adj, W, a_1, a_2):
    """Full-input entry point: shards by head across 8 cores, returns [S, H*FOUT]."""
    global LAST_RESULTS
    X = np.asarray(input_seq)[0]          # [S, FIN] f32
    adjm = np.asarray(adj)[0]             # [S, S] int32
    Wn = np.asarray(W)                    # [H, FIN, FOUT]
    a1n = np.asarray(a_1)                 # [H, FOUT, 1]
    a2n = np.asarray(a_2)                 # [H, FOUT, 1]

    s = X.shape[0]
    xt = np.ascontiguousarray(X.T, dtype=np.float16)
    # mask encoded as {0, -BIG}: p = relu(p + mask') zeroes masked-out entries
    adjt = np.where(np.ascontiguousarray(adjm.T) != 0, np.float16(0.0), np.float16(-60000.0))

    nc = _get_nc(s)
    in_maps = [
        {
            "xt": xt,
            "w": np.ascontiguousarray(Wn[h], dtype=np.float16),
            "wt": np.ascontiguousarray(Wn[h].T, dtype=np.float16),
            "a12": np.ascontiguousarray(
                np.concatenate([a1n[h], a2n[h]], axis=1), dtype=np.float16
            ),
            "adjt": adjt,
        }
        for h in range(H)
    ]
    res = bass_utils.run_bass_kernel_spmd(nc, in_maps, core_ids=list(range(H)))
    LAST_RESULTS = res

    outf = np.empty((s, H * FOUT), dtype=np.float32)
    for h in range(H):
        outf[:, h * FOUT : (h + 1) * FOUT] = res.results[h]["out"].T
    return outf



# revision 23
# speedup vs baseline: 1.0974x; 1.0170x over previous
"""GAT multi-head attention layer (nn_GATMutiHeadAttLayer) on 8 Trainium2 cores.

Head-sharded: core h computes head h entirely (no collectives).

Math (per head):
  h = X @ W                       [S, FOUT]
  f1 = h @ a1, f2 = h @ a2        [S]
  e[i,j] = lrelu(f1[i] + f2[j], 0.2), masked by adj[i,j]
  attn = softmax(e, axis=i)  (denominator s[j] = sum_i)
  out = attn @ h, concat heads, ELU.

Device formulation (transposed, j on partitions; fp16 tiles).  All per-band
elementwise ops are chosen for the DVE's fast uop modes (TS = 2 results/cyc
with per-partition scalar pair; TT = 2 results/cyc; STT/TTR are 1x and
avoided):
  exp(lrelu(z)) = max(exp(z), exp(0.2 z)),  z = f1[i] + f2[j]
  u'[j,i] = max(R1b[i] * eu[j], ev[j])          (TS: mult, max)
     R1b = exp(0.8 f1) bcast, eu = exp(f2), ev = exp(0.2 f2)
  q[j,i]  = u' * E1sb[i]                        (TT: mult)
     E1sb = exp(0.2 f1) bcast;  q = exp(lrelu(z)) unmasked, >= 0
  pb = q + mask'  (mask' in {0, -BIG} fp16, DMA'd)   (TT: add)
  p = relu(pb), s[j] = sum_i p   (one ACT pass: zeroes masked entries AND
                                  row-sums via the fused accumulator)
  hp = h[band] * (1/s)  (DVE reciprocal + ACT copy-with-scale; deferred one
                         band so the DVE FIFO never stalls on the ACT pass)
  out^T[o,i] = sum_j hp[j,o] * p[j,i]   (PE, PSUM-accumulated over bands)
  final: ELU(out^T) -> DRAM; host transposes/concats heads.

Preamble (PE fp16):
  wa = W @ [a1|a2] via W^T; zb[p,i] = f1[i] directly via a stride-0
  broadcast stationary (wa1 replicated); R1b/E1sb = Exp(0.8/0.2 * zb)
  straight out of PSUM.  [h_band | f2_band] = xt_band^T @ [W | wa2]
  (one N=65 matmul per band);  eu = exp(f2), ev = exp(0.2 f2).

Host prep: X^T, W, W^T, [a1|a2] cast fp16; adj^T encoded {0, -60000} fp16.
All model compute (matmuls, exp, masking, softmax, ELU) runs on device.

Rejected on measurement: STT/TTR formulations (1x uop, 4.4us/band), SWDGE
accumulate-DMA masking (runtime crash), GPSIMD elementwise offload (shared
SBUF port degrades DVE fast modes 2-4x), GPSIMD normalize_recip (2us/call
on the critical chain).
"""

import contextlib
import ctypes
import os
import sys
import types
from contextlib import ExitStack

import numpy as np

import concourse.bass as bass
import concourse.tile as tile
from concourse import bacc, mybir
from concourse import bass_utils

AF = mybir.ActivationFunctionType
ALU = mybir.AluOpType
DT = mybir.dt

S = 4096
FIN = 128
FOUT = 64
H = 8
ALPHA = 0.2

LAST_RESULTS = None  # BassKernelResults of the most recent run (for test harness)

# ---------------------------------------------------------------------------
# NTFF profile hook shim: antenv.axon_hooks is absent in this container; the
# trace=True path of run_bass_kernel_spmd imports it. Recreate it via ctypes
# against libaxon_pjrt.so (same as trn_agent_boot does).
_SO_PATH = "/opt/axon/libaxon_pjrt.so"


def _make_ntff_hook():
    try:
        lib = ctypes.CDLL(_SO_PATH)
    except OSError:
        return None
    if not hasattr(lib, "axon_start_nrt_profile"):
        return None
    lib.axon_start_nrt_profile.argtypes = [ctypes.POINTER(ctypes.c_int64), ctypes.c_size_t]
    lib.axon_start_nrt_profile.restype = ctypes.c_int64
    lib.axon_stop_nrt_profile.argtypes = [ctypes.c_char_p]
    lib.axon_stop_nrt_profile.restype = ctypes.c_int64

    @contextlib.contextmanager
    def _hook(output_dir, device_ids):
        import jax

        jax.devices()
        if device_ids:
            ids = (ctypes.c_int64 * len(device_ids))(*device_ids)
            rc = lib.axon_start_nrt_profile(ids, len(device_ids))
        else:
            rc = lib.axon_start_nrt_profile(None, 0)
        if rc != 0:
            raise RuntimeError(f"axon_start_nrt_profile rc={rc}")
        try:
            yield
        finally:
            n = lib.axon_stop_nrt_profile(str(output_dir).encode())
            if n <= 0:
                print(f"ntff profile: rc={n} (no files?) dir={output_dir}", file=sys.stderr)

    return _hook


def _install_ntff_shim():
    if "antenv.axon_hooks" in sys.modules:
        return
    mod = types.ModuleType("antenv.axon_hooks")
    _hook = _make_ntff_hook()
    mod.get_axon_ntff_profile_hook = lambda: _hook
    mod.set_axon_ntff_profile_hook = lambda h: None
    sys.modules["antenv.axon_hooks"] = mod
    try:
        import antenv

        antenv.axon_hooks = mod
    except ImportError:
        pass


_install_ntff_shim()

# ---------------------------------------------------------------------------

PLAN = os.environ.get("KERNEL_PLAN", "Z")
USE_GP = os.environ.get("KERNEL_GP", "0") == "1"
# Width of the per-band mask-add slice offloaded to GPSIMD (0 = all on DVE)
GPW = int(os.environ.get("KERNEL_GPW", "0"))
# Number of B-type (ACT-heavy) bands out of nb=32.  A-band: DVE TS+TT+TT,
# ACT Relu+accum.  B-band: ACT Prelu+Exp+accum, DVE mask-TT only.
# Measured per-band costs: A = 6.29us DVE / 3.97us ACT; B = 2.68 / 7.66.
# Balance lands at ~10/32.  B bands live in [2, nb-6]: none at the start
# (ramp: prelu would queue behind the preamble exps) and none at the end
# (drain: the final A-run lets ACT catch up before the last band's chain).
NB_B = int(os.environ.get("KERNEL_NB", "10"))


def build_nc(s=S, plan=None):
    """Build + compile the per-core Bass program (same program on all cores)."""
    plan = plan or PLAN
    nb = s // 128     # number of j-bands
    nch = s // 512    # number of 512-wide i-chunks

    nc = bacc.Bacc("TRN2", target_bir_lowering=False, debug=False, enable_asserts=False)

    xt = nc.dram_tensor("xt", [FIN, s], DT.float16, kind="ExternalInput").ap()
    w = nc.dram_tensor("w", [FIN, FOUT], DT.float16, kind="ExternalInput").ap()
    wt = nc.dram_tensor("wt", [FOUT, FIN], DT.float16, kind="ExternalInput").ap()
    a12 = nc.dram_tensor("a12", [FOUT, 2], DT.float16, kind="ExternalInput").ap()
    adjt = nc.dram_tensor("adjt", [s, s], DT.float16, kind="ExternalInput").ap()
    out = nc.dram_tensor("out", [FOUT, s], DT.float16, kind="ExternalOutput").ap()

    with tile.TileContext(nc) as tc, ExitStack() as ctx:
        _body(ctx, tc, nc, xt, w, wt, a12, adjt, out, s, nb, nch, plan)

    if os.environ.get("KERNEL_LDW1", "1") == "1":
        # Mark matmuls whose stationary operand AP repeats the immediately
        # preceding matmul's as non-self-loading (PE keeps the loaded array).
        n_marked = 0
        for blk in nc.m.functions[0].blocks:
            prev_w = None
            for inst in blk.instructions:
                if type(inst).__name__ != "InstMatmult":
                    continue
                wkey = repr(inst.ins[1])
                if prev_w == wkey:
                    inst.ldweights = False
                    n_marked += 1
                prev_w = wkey
        print(f"KERNEL_LDW1: marked {n_marked} matmuls non-self-loading")

    nc.compile()
    return nc


def _body(ctx, tc, nc, xt, w, wt, a12, adjt, out, s, nb, nch, plan):
    f32, f16 = DT.float32, DT.float16

    if NB_B > 1:
        bset = {2 + round(k * (nb - 8) / (NB_B - 1)) for k in range(NB_B)}
    else:
        bset = {nb // 2} if NB_B else set()

    def is_b(b):
        """True if band b runs the ACT-heavy path."""
        return b in bset

    # ---------------- persistent intermediates (live through main loop) ----
    cpool = ctx.enter_context(tc.tile_pool(name="const", bufs=1))
    r1b_sb = cpool.tile([128, s], f16, tag="r1b")      # exp(0.8 f1[i]) bcast (A bands)
    e1sb_sb = cpool.tile([128, s], f16, tag="e1sb")    # exp(0.2 f1[i]) bcast (A bands)
    f1b_sb = cpool.tile([128, s], f16, tag="f1b")      # f1[i] bcast (B bands)
    hf_sb = cpool.tile([128, nb * (FOUT + 1)], f32, tag="hf")  # [h|f2] per band
    eu_sb = cpool.tile([128, nb], f32, tag="eu")       # exp(f2), band b in col b
    ev_sb = cpool.tile([128, nb], f32, tag="ev")       # exp(0.2 f2)

    def h_col(b):
        return hf_sb[:, b * (FOUT + 1) : b * (FOUT + 1) + FOUT]

    def f2_col(b):
        return hf_sb[:, b * (FOUT + 1) + FOUT : (b + 1) * (FOUT + 1)]

    # ---------------- pools (created early: bands 0/1 are emitted inside
    # the preamble so their tiles must already exist) ----------------------
    upool = ctx.enter_context(tc.tile_pool(name="umax", bufs=2))
    ppool = ctx.enter_context(tc.tile_pool(name="pmat", bufs=4))
    scrpool = ctx.enter_context(tc.tile_pool(name="scr", bufs=3))
    spool = ctx.enter_context(tc.tile_pool(name="svec", bufs=12))
    hppool = ctx.enter_context(tc.tile_pool(name="hp", bufs=3))
    mpool = ctx.enter_context(tc.tile_pool(name="mask", bufs=4))
    qpool = ctx.enter_context(tc.tile_pool(name="qtmp", bufs=2))
    wpool = ctx.enter_context(tc.tile_pool(name="wlr", bufs=2))

    def mask_dma(b, nsplit=1):
        # nsplit>1 cuts the on-queue latency (1MB moves as 128 serial 8KB
        # packets, ~40us on one hw queue) at ~0.6us dispatch each — only
        # worth it for the ramp-gating first masks.
        m_t = mpool.tile([128, s], f16, tag="m")
        pw = 128 // nsplit
        for pg in range(nsplit):
            nc.sync.dma_start(m_t[pw * pg : pw * (pg + 1), :],
                              adjt[b * 128 + pw * pg : b * 128 + pw * (pg + 1), :])
        return m_t

    # ---------------- preamble (scoped pools, freed before main loop) ------
    # Bands 0 and 1 are processed column-chunked INSIDE the bcast loop: each
    # 1024-col chunk of TS/TT/TT is emitted right after the chunk of
    # r1b/e1sb it consumes, so the DVE starts ~15us earlier than it would
    # waiting for the full preamble (engine queues execute in emission order).
    band01 = {}
    with tc.tile_pool(name="pre_sb", bufs=1) as tpool:
        # weights first: tiny transfers that gate the wa matmul
        w65_sb = tpool.tile([FIN, FOUT + 1], f16, tag="w65")
        nc.sync.dma_start(w65_sb[:, 0:FOUT], w[:])
        wt_sb = tpool.tile([FOUT, FIN], f16, tag="wt")
        nc.sync.dma_start(wt_sb[:], wt[:])
        a12_sb = tpool.tile([FOUT, 2], f16, tag="a12")
        nc.sync.dma_start(a12_sb[:], a12[:])
        xt_sb = tpool.tile([FIN, s], f16, tag="xt")
        # split across partition-groups x column-halves: a single 1MB DMA
        # moves ~128 x 8KB packets serially on ONE hw queue (~26.5GB/s,
        # ~40us latency); 8 slices land in parallel on separate queues.
        # Half dispatched from the ACT engine to halve dispatch serialization.
        for pg in range(4):
            for ch in range(2):
                eng = nc.scalar if ch == 0 else nc.sync
                eng.dma_start(
                    xt_sb[32 * pg : 32 * (pg + 1), bass.ts(ch, s // 2)],
                    xt[32 * pg : 32 * (pg + 1), bass.ts(ch, s // 2)],
                )
        m_tiles = {0: mask_dma(0, nsplit=4), 1: mask_dma(1, nsplit=4),
                   2: mask_dma(2, nsplit=2)}
        wa_sb = tpool.tile([FIN, 2], f16, tag="wa")    # [wa1 | wa2]

        # wa = W @ [a1 | a2]  (contract over FOUT)
        with tc.tile_pool(name="pre_wa", bufs=1, space="PSUM") as pwa:
            wa_ps = pwa.tile([FIN, 2], f32, tag="wa")
            nc.tensor.matmul(wa_ps[:], lhsT=wt_sb[:], rhs=a12_sb[:], start=True, stop=True)
            nc.vector.tensor_copy(wa_sb[:], wa_ps[:])
            nc.vector.tensor_copy(w65_sb[:, FOUT : FOUT + 1], wa_ps[:, 1:2])

        # h/f2/eu/ev for bands 0/1 (tiny ops; gate the fused chunks below)
        with tc.tile_pool(name="pre_h0", bufs=2, space="PSUM") as ph0:
            for b in range(2):
                ph = ph0.tile([128, FOUT + 1], f32, tag="hf0")
                nc.tensor.matmul(ph[:], lhsT=xt_sb[:, bass.ts(b, 128)], rhs=w65_sb[:], start=True, stop=True)
                nc.vector.tensor_copy(hf_sb[:, bass.ts(b, FOUT + 1)], ph[:])
                nc.scalar.activation(eu_sb[:, b : b + 1], f2_col(b), AF.Exp)
                nc.scalar.activation(ev_sb[:, b : b + 1], f2_col(b), AF.Exp, scale=0.2)
            for b in range(2):
                u_t = upool.tile([128, s], f16, tag="u")
                q_t = qpool.tile([128, s], f16, tag="q")
                pb_t = ppool.tile([128, s], f16, tag="pb")
                p_t = scrpool.tile([128, s], f16, tag="p")
                s_t = spool.tile([128, 1], f32, tag="s")
                band01[b] = dict(u=u_t, q=q_t, pb=pb_t, p=p_t, s=s_t)

        # zb[p, i] = f1[i] for all partitions p via a stride-0 broadcast
        # stationary (wa1 replicated across 128 array columns).  Exp(0.8 z)
        # and Exp(0.2 z) feed the A bands straight out of PSUM (no DVE
        # cost); the raw f16 copy of z feeds the B bands.
        wa1rep = wa_sb[:, 0:1].broadcast_to([FIN, 128])
        with tc.tile_pool(name="pre_bc", bufs=2, space="PSUM") as pbcp:
            for c in range(s // 1024):
                lo = c * 1024
                sl = bass.ts(c, 1024)
                pbc = pbcp.tile([128, 1024], f32, tag="bc")
                for k in range(2):
                    nc.tensor.matmul(pbc[:, bass.ts(k, 512)], lhsT=wa1rep,
                                     rhs=xt_sb[:, lo + k * 512 : lo + (k + 1) * 512],
                                     start=True, stop=True)
                nc.scalar.activation(r1b_sb[:, sl], pbc[:], AF.Exp, scale=0.8)
                nc.scalar.activation(e1sb_sb[:, sl], pbc[:], AF.Exp, scale=0.2)
                for b in range(2):
                    t = band01[b]
                    nc.vector.tensor_scalar(
                        out=t["u"][:, sl], in0=r1b_sb[:, sl], scalar1=eu_sb[:, b : b + 1],
                        scalar2=ev_sb[:, b : b + 1], op0=ALU.mult, op1=ALU.max,
                    )
                    nc.vector.tensor_tensor(out=t["q"][:, sl], in0=t["u"][:, sl],
                                            in1=e1sb_sb[:, sl], op=ALU.mult)
                    nc.vector.tensor_tensor(out=t["pb"][:, sl], in0=t["q"][:, sl],
                                            in1=m_tiles[b][:, sl], op=ALU.add)
                nc.vector.tensor_copy(f1b_sb[:, sl], pbc[:])
        for b in range(2):
            t = band01[b]
            nc.scalar.activation(t["p"][:], t["pb"][:], AF.Relu, accum_out=t["s"][:])

        # [h_band | f2_band] = xt_band^T @ [W | wa2]  (one matmul + one copy
        # per band into the combined hf tile; f2 stays at stride FOUT+1)
        with tc.tile_pool(name="pre_h", bufs=3, space="PSUM") as phf:
            for b in range(2, nb):
                ph = phf.tile([128, FOUT + 1], f32, tag="hf")
                nc.tensor.matmul(ph[:], lhsT=xt_sb[:, bass.ts(b, 128)], rhs=w65_sb[:], start=True, stop=True)
                nc.vector.tensor_copy(hf_sb[:, bass.ts(b, FOUT + 1)], ph[:])

        # exp of f2 cols for bands 2+ (strided reads of hf; tiny ACT ops)
        off = 2 * (FOUT + 1) + FOUT
        f2_strided = hf_sb[:, off :: FOUT + 1]
        nc.scalar.activation(eu_sb[:, 2:nb], f2_strided, AF.Exp)
        nc.scalar.activation(ev_sb[:, 2:nb], f2_strided, AF.Exp, scale=0.2)

    # ---------------- main loop over j-bands ----------------
    mainpsum = ctx.enter_context(tc.tile_pool(name="out_psum", bufs=1, space="PSUM"))
    psum_out = mainpsum.tile([FOUT, s], f32, tag="out")

    def finish_band(b, p_t, s_t, elu_emit=None):
        """Reciprocal + hp scaling + accumulating matmuls for band b.

        Deferred so the DVE FIFO's reciprocal never waits on the ACT passes
        of the same band (head-of-line stall)."""
        hp_t = hppool.tile([128, FOUT], f16, tag="hp")
        rs_t = spool.tile([128, 1], f32, tag="rs")
        nc.vector.reciprocal(rs_t[:], s_t[:])
        # hp = h * (1/s): per-partition scalar mult on DVE (cheap TS) keeps
        # the ACT queue free for the big relu/prelu/exp passes.
        nc.vector.tensor_scalar(out=hp_t[:], in0=h_col(b), scalar1=rs_t[:],
                                scalar2=None, op0=ALU.mult)
        for c in range(nch):
            nc.tensor.matmul(
                psum_out[:, bass.ts(c, 512)], lhsT=hp_t[:], rhs=p_t[:, bass.ts(c, 512)],
                start=(b == 0), stop=(b == nb - 1),
            )
            if elu_emit is not None:
                elu_emit(c)

    # w tiles (Prelu of f1+f2) for B bands, computed one band ahead
    w_tiles = {}

    def emit_prelu(b):
        w_t = wpool.tile([128, s], f16, tag="w")
        nc.scalar.activation(w_t[:], f1b_sb[:], AF.Prelu, bias=f2_col(b), alpha=0.2)
        w_tiles[b] = w_t

    pendings = [(0, band01[0]["p"], band01[0]["s"]),
                (1, band01[1]["p"], band01[1]["s"])]
    if is_b(2):
        emit_prelu(2)
    for b in range(2, nb):
        if b in m_tiles:
            m_t = m_tiles.pop(b)
        else:
            m_t = mask_dma(b)
        p_t = scrpool.tile([128, s], f16, tag="p")
        s_t = spool.tile([128, 1], f32, tag="s")

        if b + 1 < nb and is_b(b + 1):
            emit_prelu(b + 1)  # ACT fills while DVE works on band b

        if not is_b(b):
            # ---- A band (DVE-heavy): u'=max(R1b*eu,ev); q=u'*E1sb; +mask;
            #      ACT Relu zeroes masked entries and row-sums.
            u_t = upool.tile([128, s], f16, tag="u")
            nc.vector.tensor_scalar(
                out=u_t[:], in0=r1b_sb[:], scalar1=eu_sb[:, b : b + 1],
                scalar2=ev_sb[:, b : b + 1], op0=ALU.mult, op1=ALU.max,
            )
            q_t = qpool.tile([128, s], f16, tag="q")
            nc.vector.tensor_tensor(out=q_t[:], in0=u_t[:], in1=e1sb_sb[:], op=ALU.mult)
            if pendings:
                finish_band(*pendings.pop(0))
            pb_t = ppool.tile([128, s], f16, tag="pb")
            nc.vector.tensor_tensor(out=pb_t[:], in0=q_t[:], in1=m_t[:], op=ALU.add)
            nc.scalar.activation(p_t[:], pb_t[:], AF.Relu, accum_out=s_t[:])
        else:
            # ---- B band (ACT-heavy): pb = prelu(f1+f2) + mask; p = exp(pb)
            w_t = w_tiles.pop(b)
            pb_t = ppool.tile([128, s], f16, tag="pb")
            nc.vector.tensor_tensor(out=pb_t[:], in0=w_t[:], in1=m_t[:], op=ALU.add)
            if pendings:
                finish_band(*pendings.pop(0))
            nc.scalar.activation(p_t[:], pb_t[:], AF.Exp, accum_out=s_t[:])

        pendings.append((b, p_t, s_t))

    # ---------------- ELU + writeout, pipelined with the last band's stop-
    # matmuls: chunk c's ELU is emitted right after its final accumulation.
    fpool = ctx.enter_context(tc.tile_pool(name="fin", bufs=2))
    ew = s // nch  # 512

    def elu_emit(c):
        sl = bass.ts(c, ew)
        r_t = fpool.tile([FOUT, ew], f32, tag="relu")
        nc.scalar.activation(r_t[:], psum_out[:, sl], AF.Relu)
        mn_t = fpool.tile([FOUT, ew], f32, tag="min")
        nc.vector.tensor_scalar_min(out=mn_t[:], in0=psum_out[:, sl], scalar1=0.0)
        e_t = fpool.tile([FOUT, ew], f32, tag="exp")
        nc.scalar.activation(e_t[:], mn_t[:], AF.Exp)
        f_t = fpool.tile([FOUT, ew], f16, tag="fin")
        # f = (e - 1) + r   (f16 out halves the writeback; host upcasts)
        nc.vector.scalar_tensor_tensor(out=f_t[:], in0=e_t[:], scalar=-1.0, in1=r_t[:], op0=ALU.add, op1=ALU.add)
        nc.sync.dma_start(out[:, sl], f_t[:])

    while len(pendings) > 1:
        finish_band(*pendings.pop(0))
    finish_band(*pendings.pop(0), elu_emit=elu_emit)


_NC_CACHE = {}


def _get_nc(s=S, plan=None):
    key = (s, plan or PLAN)
    if key not in _NC_CACHE:
        _NC_CACHE[key] = build_nc(s, plan)
    return _NC_CACHE[key]


def kernel(input_seq, adj, W, a_1, a_2):
    """Full-input entry point: shards by head across 8 cores, returns [S, H*FOUT]."""
    global LAST_RESULTS
    X = np.asarray(input_seq)[0]          # [S, FIN] f32
    adjm = np.asarray(adj)[0]             # [S, S] int32
    Wn = np.asarray(W)                    # [H, FIN, FOUT]
    a1n = np.asarray(a_1)                 # [H, FOUT, 1]
    a2n = np.asarray(a_2)                 # [H, FOUT, 1]

    s = X.shape[0]
    xt = np.ascontiguousarray(X.T, dtype=np.float16)
    # mask encoded as {0, -BIG}: p = relu(p + mask') zeroes masked-out entries
    adjt = np.where(np.ascontiguousarray(adjm.T) != 0, np.float16(0.0), np.float16(-60000.0))

    nc = _get_nc(s)
    in_maps = [
        {
            "xt": xt,
            "w": np.ascontiguousarray(Wn[h], dtype=np.float16),
            "wt": np.ascontiguousarray(Wn[h].T, dtype=np.float16),
            "a12": np.ascontiguousarray(
                np.concatenate([a1n[h], a2n[h]], axis=1), dtype=np.float16
            ),
            "adjt": adjt,
        }
        for h in range(H)
    ]
    res = bass_utils.run_bass_kernel_spmd(nc, in_maps, core_ids=list(range(H)))
    LAST_RESULTS = res

    outf = np.empty((s, H * FOUT), dtype=np.float32)
    for h in range(H):
        outf[:, h * FOUT : (h + 1) * FOUT] = res.results[h]["out"].T
    return outf

